# revision 3
# baseline (speedup 1.0000x reference)
"""MoE dispatcher kernel for Trainium2 (8 NeuronCores, expert-parallel).

Contract: kernel(**inputs) takes FULL inputs and returns the FULL output.

Strategy (expert-parallel, one expert per core):
  - host: softmax(gate_logits) -> top-2 -> combine weights per (token, expert)
  - host "all-to-all dispatch": for expert e, gather its routed tokens,
    pre-scale rows by the combine weight (w * (x @ W) == (w*x) @ W), pad to a
    common capacity C, transpose to [D, C] so the device streams tokens along
    the free dim.  One expert per core.
  - device (per core): Y^T[D,C] = W[e]^T @ X^T via PE array.  Loop nest is
    (m-tile-pair, k, m-in-pair, n-tile) so that:
      * X arrives in k-major chunks -> the first pair of m-tiles starts after
        ~2us of DMA and is DMA-paced (absorbing the HAM cold-clock ramp),
      * W arrives in one 512KB chunk per m-tile-pair, in use order,
      * one LDWEIGHTS covers all n-tiles of an (k, m) step,
      * PSUM: 6 banks per pair, pool rotation overlaps pair boundaries.
  - PSUM evicted per pair into a [P, 2*C] bf16 staging tile, one output DMA
    per pair (bf16 halves the write traffic; rel-err budget is 2e-2).
  - host "all-to-all combine": scatter-add each expert's Y rows back to the
    token axis (plain add; weights were folded into x).

DRAM layouts (host-permuted so every DMA is contiguous per partition):
  x   [P, KT*C]        x[p, k*C + n]                 = X^T[k*128 + p, n]
  w   [P, MT/2*KT*256] w[p, ((pr*KT)+k)*256 + j2*128 + j]
                        = W[e][k*128 + p, (pr*2 + j2)*128 + j]
  yt  [MT/2, P, 2*C]   yt[pr, p, j2*C + n]           = Y^T[(pr*2+j2)*128+p, n]
"""

import os

import numpy as np

N_CORES = 8
P = 128
NSPLIT = 512  # PSUM bank / max fp32 moving free dim
WARMUP_MM = int(os.environ.get("BASS_MOE_WARMUP", "4"))
X_CHUNK_K = int(os.environ.get("BASS_MOE_XCHUNK", "2"))  # k-tiles per X DMA

_prog_cache: dict = {}


def _np_bf16():
    import ml_dtypes

    return ml_dtypes.bfloat16


def _n_tiles(C):
    """Split C into column tiles of at most NSPLIT (one PSUM bank each)."""
    out = []
    n0 = 0
    while n0 < C:
        sz = min(NSPLIT, C - n0)
        out.append((n0, sz))
        n0 += sz
    return out


def _build_program(D: int, C: int):
    import concourse.bacc as bacc
    import concourse.mybir as mybir
    import concourse.tile as tile

    bf16 = mybir.dt.bfloat16
    f32 = mybir.dt.float32
    KT = D // P  # contraction tiles
    MT = D // P  # output-feature tiles
    NPR = MT // 2  # m-tile pairs
    n_tiles = _n_tiles(C)
    NT = len(n_tiles)
    assert 2 * NT <= 8, "PSUM banks: need 2*NT <= 8"
    assert KT % X_CHUNK_K == 0

    nc = bacc.Bacc(None, target_bir_lowering=False)
    x = nc.declare_dram_parameter("x", [P, KT * C], bf16, isOutput=False)
    w = nc.declare_dram_parameter("w", [P, NPR * KT * 256], bf16, isOutput=False)
    yt = nc.declare_dram_parameter("yt", [NPR, P, 2 * C], bf16, isOutput=True)

    with tile.TileContext(nc) as tc:
        with (
            tc.tile_pool(name="xpool", bufs=1) as xpool,
            tc.tile_pool(name="wpool", bufs=NPR) as wpool,
            tc.tile_pool(name="psum", bufs=8, space="PSUM") as psum_pool,
            tc.tile_pool(name="stage", bufs=2) as stpool,
            tc.tile_pool(name="warm", bufs=1) as warmpool,
        ):
            # Keep the PE busy during the DMA lead-in (HAM clock-gate warmup).
            if WARMUP_MM:
                wt = warmpool.tile([P, NSPLIT], bf16, tag="warm_w")
                nc.vector.memset(wt[:], 0.0)
                for _ in range(WARMUP_MM):
                    wp = psum_pool.tile([P, NSPLIT], f32, tag="ps")
                    nc.tensor.matmul(
                        wp[:], lhsT=wt[:, :P], rhs=wt[:], start=True, stop=True
                    )

            # Input DMAs, issued in consumption order.  W on the sync HWDGE
            # queue, X (+ output) on the scalar HWDGE queue — independent
            # FIFOs, so X chunks are not stuck behind W chunks.
            w_sb = []
            for pr in range(NPR):
                twl = wpool.tile([P, KT, 256], bf16, tag="w_sb")
                nc.sync.dma_start(
                    twl[:].rearrange("p k j -> p (k j)"),
                    w[:, pr * KT * 256 : (pr + 1) * KT * 256],
                )
                w_sb.append(twl)
            x_sb = xpool.tile([P, KT, C], bf16, tag="x_sb")
            for c in range(KT // X_CHUNK_K):
                k0 = c * X_CHUNK_K
                nc.scalar.dma_start(
                    x_sb[:, k0 : k0 + X_CHUNK_K, :].rearrange("p k n -> p (k n)"),
                    x[:, k0 * C : (k0 + X_CHUNK_K) * C],
                )

            for pr in range(NPR):
                ps = [
                    [
                        psum_pool.tile([P, NSPLIT], f32, tag="ps", name="ps")
                        for _ in n_tiles
                    ]
                    for _ in range(2)
                ]
                for k in range(KT):
                    for j2 in range(2):
                        lhsT = w_sb[pr][:, k, j2 * P : (j2 + 1) * P]
                        for t, (n0, nsz) in enumerate(n_tiles):
                            nc.tensor.matmul(
                                ps[j2][t][:, :nsz],
                                lhsT=lhsT,
                                rhs=x_sb[:, k, n0 : n0 + nsz],
                                start=(k == 0),
                                stop=(k == KT - 1),
                            )
                stage = stpool.tile([P, 2 * C], bf16, tag="stage")
                for j2 in range(2):
                    for t, (n0, nsz) in enumerate(n_tiles):
                        nc.vector.tensor_copy(
                            stage[:, j2 * C + n0 : j2 * C + n0 + nsz],
                            ps[j2][t][:, :nsz],
                        )
                nc.scalar.dma_start(yt[pr, :, :], stage[:])
    nc.compile()
    return nc


def kernel(hidden: np.ndarray, gate_logits: np.ndarray, W: np.ndarray) -> np.ndarray:
    from concourse.bass_utils import run_bass_kernel_spmd

    hidden = np.asarray(hidden)
    gate_logits = np.asarray(gate_logits)
    W = np.asarray(W)
    B, S, D = hidden.shape
    T, E = gate_logits.shape
    assert E == N_CORES
    KT = D // P
    MT = D // P
    NPR = MT // 2
    bf16 = _np_bf16()
    x = np.ascontiguousarray(hidden.reshape(T, D).astype(np.float32))

    # --- routing on host (fp32, matches reference softmax/top-2) ---
    g = gate_logits.astype(np.float32)
    m = g.max(axis=-1, keepdims=True)
    p = np.exp(g - m)
    p /= p.sum(axis=-1, keepdims=True)
    top2 = np.argpartition(-p, 1, axis=-1)[:, :2]

    routed = [np.nonzero((top2 == e).any(axis=1))[0] for e in range(E)]
    counts = np.array([len(r) for r in routed])
    C = max(NSPLIT, int(-(-counts.max() // P)) * P)  # capacity, multiple of 128

    in_maps = []
    for e in range(E):
        idx = routed[e]
        cnt = len(idx)
        scale = p[idx, e].astype(np.float32)
        xe = (x[idx] * scale[:, None]).astype(bf16)  # [cnt, D]
        # x dram [P, KT*C]: x[p, k*C+n] = Xe^T[k*128+p, n]
        xk = np.zeros((KT, P, C), dtype=bf16)
        xk[:, :, :cnt] = xe.T.reshape(KT, P, cnt)
        x_dram = np.ascontiguousarray(xk.transpose(1, 0, 2).reshape(P, KT * C))
        # w dram [P, NPR*KT*256]: w[p, (pr*KT+k)*256 + j2*128 + j]
        #   = W[e][k*128+p, (pr*2+j2)*128+j]
        Wb = W[e].astype(bf16).reshape(KT, P, MT, P)
        w_dram = np.ascontiguousarray(
            Wb.transpose(1, 2, 0, 3)  # [p, mi, k, j]
            .reshape(P, NPR, 2, KT, P)
            .transpose(0, 1, 3, 2, 4)  # [p, pr, k, j2, j]
            .reshape(P, NPR * KT * 256)
        )
        in_maps.append({"x": x_dram, "w": w_dram})

    key = (D, C)
    if key not in _prog_cache:
        _prog_cache[key] = _build_program(D, C)
    nc = _prog_cache[key]

    res = run_bass_kernel_spmd(nc, in_maps, core_ids=list(range(N_CORES)))

    # --- combine on host ---
    out = np.zeros((T, D), dtype=np.float32)
    for e in range(E):
        idx = routed[e]
        cnt = len(idx)
        # yt [NPR, P, 2*C] -> Y^T [D, C] with row m = (pr*2+j2)*128 + p
        ye = np.asarray(res.results[e]["yt"])
        ye_t = ye.reshape(NPR, P, 2, C).transpose(0, 2, 1, 3).reshape(D, C)
        out[idx] += ye_t[:, :cnt].T.astype(np.float32)
    return out.reshape(B, S, D)


# revision 7
# speedup vs baseline: 1.0379x; 1.0379x over previous
"""MoE dispatcher kernel for Trainium2 (8 NeuronCores, expert-parallel).

Contract: kernel(**inputs) takes FULL inputs and returns the FULL output.

Strategy (expert-parallel, one expert per core):
  - host: softmax(gate_logits) -> top-2 -> combine weights per (token, expert)
  - host "all-to-all dispatch": for expert e, gather its routed tokens,
    pre-scale rows by the combine weight (w * (x @ W) == (w*x) @ W), pad to a
    common capacity C, transpose to [D, C] so the device streams tokens along
    the free dim.  One expert per core.
  - device (per core): Y^T[D,C] = W[e]^T @ X^T via PE array.  Loop nest is
    (m-tile-pair, k, m-in-pair, n-tile) so that:
      * X arrives in k-major chunks -> the first pair of m-tiles starts after
        ~1.5us of DMA and is DMA-paced (absorbing the HAM cold-clock ramp),
      * W arrives in one chunk per m-tile-pair, in use order,
      * one LDWEIGHTS covers all n-tiles of a (k, m) step,
      * warmup matmuls (kept live via a tiny debug output) fill the PE while
        the first X/W chunks land.
  - PSUM evicted per (pair, m) into a [P, C] bf16 staging tile, one output
    DMA per (pair, j2) (bf16 halves write traffic; rel-err budget is 2e-2).
  - host "all-to-all combine": scatter-add each expert's Y rows back to the
    token axis (plain add; weights were folded into x).

DRAM layouts (host-permuted so every DMA is contiguous per partition):
  x   [P, KT*C]        x[p, k*C + n]                 = X^T[k*128 + p, n]
  w   [P, MT/2*KT*256] w[p, ((pr*KT)+k)*256 + j2*128 + j]
                        = W[e][k*128 + p, (pr*2 + j2)*128 + j]
  yt  [MT, P, C]       yt[mi, p, n]                  = Y^T[mi*128 + p, n]
"""

import os

import numpy as np

N_CORES = 8
P = 128
NSPLIT = 512  # PSUM bank / max fp32 moving free dim
WARMUP_MM = int(os.environ.get("BASS_MOE_WARMUP", "8"))
X_CHUNK_K = int(os.environ.get("BASS_MOE_XCHUNK", "1"))  # k-tiles per X DMA
W0_SPLIT = int(os.environ.get("BASS_MOE_W0SPLIT", "2"))  # chunks for pair-0 W

_prog_cache: dict = {}


def _np_bf16():
    import ml_dtypes

    return ml_dtypes.bfloat16


def _n_tiles(C):
    """Split C into column tiles of at most NSPLIT (one PSUM bank each)."""
    out = []
    n0 = 0
    while n0 < C:
        sz = min(NSPLIT, C - n0)
        out.append((n0, sz))
        n0 += sz
    return out


def _build_program(D: int, C: int):
    import concourse.bacc as bacc
    import concourse.mybir as mybir
    import concourse.tile as tile

    bf16 = mybir.dt.bfloat16
    f32 = mybir.dt.float32
    KT = D // P  # contraction tiles
    MT = D // P  # output-feature tiles
    NPR = MT // 2  # m-tile pairs
    n_tiles = _n_tiles(C)
    NT = len(n_tiles)
    assert 2 * NT <= 8, "PSUM banks: need 2*NT <= 8"
    assert KT % X_CHUNK_K == 0 and KT % W0_SPLIT == 0

    nc = bacc.Bacc(None, target_bir_lowering=False)
    x = nc.declare_dram_parameter("x", [P, KT * C], bf16, isOutput=False)
    w = nc.declare_dram_parameter("w", [P, NPR * KT * 256], bf16, isOutput=False)
    yt = nc.declare_dram_parameter("yt", [MT, P, C], bf16, isOutput=True)
    dbg = nc.declare_dram_parameter("dbg", [1, 8], f32, isOutput=True)

    with tile.TileContext(nc) as tc:
        with (
            tc.tile_pool(name="xpool", bufs=1) as xpool,
            tc.tile_pool(name="wpool", bufs=NPR) as wpool,
            tc.tile_pool(name="psum", bufs=8, space="PSUM") as psum_pool,
            tc.tile_pool(name="stage", bufs=4) as stpool,
            tc.tile_pool(name="warm", bufs=2) as warmpool,
        ):
            # Warmup matmuls: keep the PE busy (and the HAM clock-gate
            # warming) while the first X/W chunks stream in.  All accumulate
            # into one PSUM tile whose first element is DMA'd to a debug
            # output, so the chain is live and survives DCE.
            if WARMUP_MM:
                wt = warmpool.tile([P, NSPLIT], bf16, tag="warm_w")
                nc.vector.memset(wt[:], 0.0)
                wp = psum_pool.tile([P, NSPLIT], f32, tag="ps", name="warm_ps")
                for i in range(WARMUP_MM):
                    nc.tensor.matmul(
                        wp[:],
                        lhsT=wt[:, :P],
                        rhs=wt[:],
                        start=(i == 0),
                        stop=(i == WARMUP_MM - 1),
                    )
                wout = warmpool.tile([1, 8], f32, tag="warm_out")
                nc.vector.tensor_copy(wout[:], wp[:1, :8])

            # Input DMAs, issued in consumption order.  W on the sync HWDGE
            # queue, X (+ output) on the scalar HWDGE queue — independent
            # FIFOs, so X chunks are not stuck behind W chunks.
            w_sb = []
            for pr in range(NPR):
                twl = wpool.tile([P, KT, 256], bf16, tag="w_sb")
                nsp = W0_SPLIT if pr == 0 else 1
                kc = KT // nsp
                for c in range(nsp):
                    nc.sync.dma_start(
                        twl[:, c * kc : (c + 1) * kc, :].rearrange(
                            "p k j -> p (k j)"
                        ),
                        w[
                            :,
                            (pr * KT + c * kc) * 256 : (pr * KT + (c + 1) * kc)
                            * 256,
                        ],
                    )
                w_sb.append(twl)
            x_sb = xpool.tile([P, KT, C], bf16, tag="x_sb")
            for c in range(KT // X_CHUNK_K):
                k0 = c * X_CHUNK_K
                nc.scalar.dma_start(
                    x_sb[:, k0 : k0 + X_CHUNK_K, :].rearrange("p k n -> p (k n)"),
                    x[:, k0 * C : (k0 + X_CHUNK_K) * C],
                )

            for pr in range(NPR):
                ps = [
                    [
                        psum_pool.tile([P, NSPLIT], f32, tag="ps", name="ps")
                        for _ in n_tiles
                    ]
                    for _ in range(2)
                ]
                for k in range(KT):
                    for j2 in range(2):
                        lhsT = w_sb[pr][:, k, j2 * P : (j2 + 1) * P]
                        for t, (n0, nsz) in enumerate(n_tiles):
                            nc.tensor.matmul(
                                ps[j2][t][:, :nsz],
                                lhsT=lhsT,
                                rhs=x_sb[:, k, n0 : n0 + nsz],
                                start=(k == 0),
                                stop=(k == KT - 1),
                            )
                for j2 in range(2):
                    stage = stpool.tile([P, C], bf16, tag="stage", name="stage")
                    for t, (n0, nsz) in enumerate(n_tiles):
                        nc.vector.tensor_copy(
                            stage[:, n0 : n0 + nsz], ps[j2][t][:, :nsz]
                        )
                    nc.scalar.dma_start(yt[pr * 2 + j2, :, :], stage[:])
            if WARMUP_MM:
                # Issue the liveness-anchor DMA last so its sem wait doesn't
                # block the X/output DMA triggers queued on the same engine.
                nc.scalar.dma_start(dbg[:, :], wout[:])
    nc.compile()
    return nc


def kernel(hidden: np.ndarray, gate_logits: np.ndarray, W: np.ndarray) -> np.ndarray:
    from concourse.bass_utils import run_bass_kernel_spmd

    hidden = np.asarray(hidden)
    gate_logits = np.asarray(gate_logits)
    W = np.asarray(W)
    B, S, D = hidden.shape
    T, E = gate_logits.shape
    assert E == N_CORES
    KT = D // P
    MT = D // P
    NPR = MT // 2
    bf16 = _np_bf16()
    x = np.ascontiguousarray(hidden.reshape(T, D).astype(np.float32))

    # --- routing on host (fp32, matches reference softmax/top-2) ---
    g = gate_logits.astype(np.float32)
    m = g.max(axis=-1, keepdims=True)
    p = np.exp(g - m)
    p /= p.sum(axis=-1, keepdims=True)
    top2 = np.argpartition(-p, 1, axis=-1)[:, :2]

    routed = [np.nonzero((top2 == e).any(axis=1))[0] for e in range(E)]
    counts = np.array([len(r) for r in routed])
    C = max(NSPLIT, int(-(-counts.max() // P)) * P)  # capacity, multiple of 128

    in_maps = []
    for e in range(E):
        idx = routed[e]
        cnt = len(idx)
        scale = p[idx, e].astype(np.float32)
        xe = (x[idx] * scale[:, None]).astype(bf16)  # [cnt, D]
        # x dram [P, KT*C]: x[p, k*C+n] = Xe^T[k*128+p, n]
        xk = np.zeros((KT, P, C), dtype=bf16)
        xk[:, :, :cnt] = xe.T.reshape(KT, P, cnt)
        x_dram = np.ascontiguousarray(xk.transpose(1, 0, 2).reshape(P, KT * C))
        # w dram [P, NPR*KT*256]: w[p, (pr*KT+k)*256 + j2*128 + j]
        #   = W[e][k*128+p, (pr*2+j2)*128+j]
        Wb = W[e].astype(bf16).reshape(KT, P, MT, P)
        w_dram = np.ascontiguousarray(
            Wb.transpose(1, 2, 0, 3)  # [p, mi, k, j]
            .reshape(P, NPR, 2, KT, P)
            .transpose(0, 1, 3, 2, 4)  # [p, pr, k, j2, j]
            .reshape(P, NPR * KT * 256)
        )
        in_maps.append({"x": x_dram, "w": w_dram})

    key = (D, C)
    if key not in _prog_cache:
        _prog_cache[key] = _build_program(D, C)
    nc = _prog_cache[key]

    res = run_bass_kernel_spmd(nc, in_maps, core_ids=list(range(N_CORES)))

    # --- combine on host ---
    out = np.zeros((T, D), dtype=np.float32)
    for e in range(E):
        idx = routed[e]
        cnt = len(idx)
        ye_t = np.asarray(res.results[e]["yt"]).reshape(D, C)  # Y^T
        out[idx] += ye_t[:, :cnt].T.astype(np.float32)
    return out.reshape(B, S, D)
